# revision 7
# baseline (speedup 1.0000x reference)
"""AttentionPairBias Trainium2 kernel (8-core SPMD, row-sharded).

Sharding: core c owns query rows i in [128c, 128c+128) and the matching z
rows z[:, i_shard, :, :]. k/v are computed replicated from the full s. The
pair-bias path computes LayerNorm stats with bn_stats, projects raw z16
through wz on the PE (per-j transposes), and folds the LN mean/rstd in as a
post-matmul correction:
    bias_h(i,j) = rs_ij * (P_raw_h(i,j) - mu_ij * c1_h)   [+ const_h, dropped:
softmax is shift-invariant per row]. z_norm_w is folded into wz, z_norm_b
drops with the constant.
"""
import numpy as np

import concourse.bass as bass
import concourse.tile as tile_mod
from concourse import mybir
from concourse.tile import TileContext
from concourse.masks import make_identity
from concourse.vector_clock import ScopedClock

F32 = mybir.dt.float32
F16 = mybir.dt.float16

S = 1024          # sequence length
DS = 1024         # model dim
H = 16            # heads
HD = 64           # head dim
DZ = 128          # pair dim
NCORES = 8
SI = S // NCORES  # 128 query rows per core


# ---------------------------------------------------------------------------
# Framework patch: this walrus build accepts only ONE semaphore wait per
# instruction, but TileContext's final drain aggregates every outstanding sem
# wait onto a single SP Drain. Split the waits across a chain of Drains.
# ---------------------------------------------------------------------------
def _patched_drain_and_barrier(self, tick_clock, wait_clock):
    nc = self.nc
    drain_inst = nc.sync.drain()
    wait_clock.add_sem_waits(
        drain_inst.ins, ScopedClock({None: tick_clock.global_clock})
    )
    si = drain_inst.ins.sync_info
    if si is not None and si.on_wait is not None and len(si.on_wait) > 1:
        extra = list(si.on_wait[1:])
        del si.on_wait[1:]
        for w in extra:
            d2 = nc.sync.drain()
            si2 = d2.ins.sync_info
            if si2 is None:
                d2.ins.sync_info = mybir.SyncInfo(on_wait=[w], on_update=[])
            else:
                si2.on_wait.append(w)
    nc.all_engine_barrier()
    assert self.sems is not None
    popped = nc._tile_sem_poison_stack.pop()
    assert popped is self._sem_poison
    nc.clear_and_free_semaphores(list(self.sems.allocated().values()))
    nc.all_engine_barrier()


def _install_patches():
    tile_mod.TileContext._drain_and_barrier = _patched_drain_and_barrier


_install_patches()


def _bn_stats_noopt(nc, out, in_):
    """bn_stats with opt=False AP lowering so batched multi-group outputs keep
    their [p, groups, 6] structure (the optimizer would collapse contiguous
    dims and the grouped write would go to the wrong place)."""
    return nc.vector.add_instruction(
        mybir.InstBNStats(
            name=nc.get_next_instruction_name(),
            ins=[nc.vector.lower_ap(in_, opt=False)],
            outs=[nc.vector.lower_ap(out, opt=False)],
        )
    )


def _bcast(ap, dims):
    """Build a broadcast view of `ap` with explicit [step, count] dims."""
    return bass.AP(tensor=ap.tensor, offset=ap.offset, ap=dims)


def build_nc(split_waits=True):
    nc = bass.Bass("TRN2", target_bir_lowering=False, debug=False,
                   num_devices=NCORES)

    z_sh = nc.dram_tensor("z_sh", [SI, S, DZ], F32, kind="ExternalInput").ap()
    sT16 = nc.dram_tensor("sT16", [DS, S], F16, kind="ExternalInput").ap()
    sTi16 = nc.dram_tensor("sTi16", [DS, SI], F16, kind="ExternalInput").ap()
    wqT16 = nc.dram_tensor("wqT16", [DS, DS], F16, kind="ExternalInput").ap()
    wkT16 = nc.dram_tensor("wkT16", [DS, DS], F16, kind="ExternalInput").ap()
    wvT16 = nc.dram_tensor("wvT16", [DS, DS], F16, kind="ExternalInput").ap()
    wgT16 = nc.dram_tensor("wgT16", [DS, DS], F16, kind="ExternalInput").ap()
    woT16 = nc.dram_tensor("woT16", [DS, DS], F16, kind="ExternalInput").ap()
    wz16 = nc.dram_tensor("wz16", [DZ, H], F16, kind="ExternalInput").ap()
    c1h = nc.dram_tensor("c1h", [1, H], F32, kind="ExternalInput").ap()
    bq8 = nc.dram_tensor("bq8", [DS, 1], F32, kind="ExternalInput").ap()
    out_sh = nc.dram_tensor("out_sh", [SI, DS], F32, kind="ExternalOutput").ap()

    with TileContext(nc) as tc:
        _emit(nc, tc, z_sh, sT16, sTi16, wqT16, wkT16, wvT16, wgT16, woT16,
              wz16, c1h, bq8, out_sh)
    if split_waits:
        _split_multiwait(nc)
    return nc


def _split_multiwait(nc):
    """This walrus build accepts at most one semaphore wait per instruction;
    Tile emits more when an op depends on producers on several engines. Hoist
    all-but-one wait onto same-engine NOPs inserted just before."""
    for fn in nc.m.functions:
        for bb in fn.blocks:
            out = []
            changed = False
            for inst in bb.instructions:
                si = inst.sync_info
                if si is not None and si.on_wait is not None and len(si.on_wait) > 1:
                    extra = list(si.on_wait[:-1])
                    del si.on_wait[:-1]
                    for w in extra:
                        out.append(mybir.InstNoOp(
                            name=nc.get_next_instruction_name(),
                            engine=inst.engine,
                            bass_nofuse=True,
                            sync_info=mybir.SyncInfo(on_wait=[w], on_update=[]),
                        ))
                    changed = True
                out.append(inst)
            if changed:
                bb.instructions[:] = out


def _emit(nc, tc, z_sh, sT16, sTi16, wqT16, wkT16, wvT16, wgT16, woT16,
          wz16, c1h, bq8, out_sh):
    from contextlib import ExitStack
    AL = mybir.AluOpType
    AF = mybir.ActivationFunctionType

    KT = 8   # 1024/128 K tiles
    G = 8    # j-group size in the z pipeline
    NG = S // G          # 128 groups
    JB = 32              # j's per P psum bank
    NB = S // JB         # 32 P banks
    RND = 256            # j's per stats-finalize round
    NR = S // RND        # 4 rounds

    ctx = ExitStack()
    with ctx:
        consts = ctx.enter_context(tc.tile_pool(name="consts", bufs=1))
        persist = ctx.enter_context(tc.tile_pool(name="persist", bufs=1))

        ident16 = consts.tile([128, 128], F16)
        make_identity(nc, ident16)
        wz_sb = consts.tile([DZ, H], F16)
        nc.sync.dma_start(out=wz_sb, in_=wz16)
        c1h_sb = consts.tile([128, H], F32)
        nc.sync.dma_start(out=c1h_sb, in_=_bcast(c1h, [[0, 128], [1, H]]))
        bq_sb = consts.tile([128, KT], F32)
        nc.sync.dma_start(out=bq_sb, in_=bq8.rearrange("(m p) o -> p (m o)", p=128))
        eps_sb = consts.tile([128, 1], F32)
        nc.vector.memset(eps_sb, 1e-5)

        # persistent SBUF tensors
        kT_sb = persist.tile([128, KT, S], F16)     # [d-part, d-tile, j]
        v_sb = persist.tile([128, KT, DS], F16)     # [j-part, j-tile, d]
        qT_sb = persist.tile([128, KT, SI], F16)    # [d-part, d-tile, i]
        g16 = persist.tile([128, DS], F16)          # [i, d]
        P16 = persist.tile([128, S, H], F16)        # [i, j, h] -> becomes bias
        st_sb = persist.tile([128, S, 6], F32)      # bn_stats out [i, j, 6]
        mu2 = persist.tile([128, S], F32)           # 2*mu
        rs = persist.tile([128, S], F32)            # 1/sqrt(var+eps)
        murs = persist.tile([128, S], F32)          # mu2*rs
        sums = persist.tile([128, H], F32)          # softmax sums
        inv = persist.tile([128, H], F32)           # 1/sums
        og16 = persist.tile([128, DS], F16)
        ogT_sb = persist.tile([128, KT, SI], F16)
        out_sb = persist.tile([128, DS], F32)

        # ---------------- Phase A: projections ----------------
        with (
            tc.tile_pool(name="wpool", bufs=1) as wpool,
            tc.tile_pool(name="apsum", bufs=2, space="PSUM") as apsum,
        ):
            sT_sb = wpool.tile([128, KT, S], F16)
            nc.sync.dma_start(
                out=sT_sb, in_=sT16.rearrange("(m p) n -> p m n", p=128))
            sTi_sb = wpool.tile([128, KT, SI], F16)
            nc.sync.dma_start(
                out=sTi_sb, in_=sTi16.rearrange("(m p) n -> p m n", p=128))
            wq_sb = wpool.tile([128, KT, DS], F16)
            nc.sync.dma_start(
                out=wq_sb, in_=wqT16.rearrange("(m p) n -> p m n", p=128))
            wk_sb = wpool.tile([128, KT, DS], F16)
            nc.sync.dma_start(
                out=wk_sb, in_=wkT16.rearrange("(m p) n -> p m n", p=128))
            wv_sb = wpool.tile([128, KT, DS], F16)
            nc.sync.dma_start(
                out=wv_sb, in_=wvT16.rearrange("(m p) n -> p m n", p=128))
            wg_sb = wpool.tile([128, KT, DS], F16)
            nc.sync.dma_start(
                out=wg_sb, in_=wgT16.rearrange("(m p) n -> p m n", p=128))

            # qT[d, i] += bq  (wq, bq pre-scaled by 1/8 on host)
            for m in range(KT):
                qp = apsum.tile([128, SI], F32, tag="qp")
                for k in range(KT):
                    nc.tensor.matmul(qp, wq_sb[:, k, 128 * m:128 * (m + 1)],
                                     sTi_sb[:, k, :],
                                     start=(k == 0), stop=(k == KT - 1))
                nc.vector.tensor_scalar(
                    out=qT_sb[:, m, :], in0=qp, scalar1=bq_sb[:, m:m + 1],
                    scalar2=None, op0=AL.add)

            # g = sigmoid(s_i @ wg^T)   [i, d]
            for n in range(2):
                gp = apsum.tile([128, 512], F32, tag="gp")
                for k in range(KT):
                    nc.tensor.matmul(gp, sTi_sb[:, k, :],
                                     wg_sb[:, k, 512 * n:512 * (n + 1)],
                                     start=(k == 0), stop=(k == KT - 1))
                nc.scalar.activation(g16[:, 512 * n:512 * (n + 1)], gp,
                                     AF.Sigmoid)

            # kT[d, j] full
            for m in range(KT):
                for n in range(2):
                    kp = apsum.tile([128, 512], F32, tag="kp")
                    for k in range(KT):
                        nc.tensor.matmul(kp, wk_sb[:, k, 128 * m:128 * (m + 1)],
                                         sT_sb[:, k, 512 * n:512 * (n + 1)],
                                         start=(k == 0), stop=(k == KT - 1))
                    nc.any.tensor_copy(kT_sb[:, m, 512 * n:512 * (n + 1)], kp)

            # v[j, d] full
            for m in range(KT):
                for n in range(2):
                    vp = apsum.tile([128, 512], F32, tag="vp")
                    for k in range(KT):
                        nc.tensor.matmul(vp, sT_sb[:, k, 128 * m:128 * (m + 1)],
                                         wv_sb[:, k, 512 * n:512 * (n + 1)],
                                         start=(k == 0), stop=(k == KT - 1))
                    nc.any.tensor_copy(v_sb[:, m, 512 * n:512 * (n + 1)], vp)

        # ---------------- Phase B: z pipeline ----------------
        with (
            tc.tile_pool(name="zpool", bufs=4) as zpool,
            tc.tile_pool(name="ztpool", bufs=3) as ztpool,
            tc.tile_pool(name="zpsum", bufs=3, space="PSUM") as zpsum,
            tc.tile_pool(name="ppsum", bufs=2, space="PSUM") as ppsum,
            tc.tile_pool(name="stmp", bufs=2) as stmp,
        ):
            pbank = None
            for jg in range(NG):
                j0 = jg * G
                z16 = zpool.tile([128, G, DZ], F16, tag="z16")
                nc.gpsimd.dma_start(out=z16, in_=z_sh[:, j0:j0 + G, :])

                # LayerNorm stats (walrus requires one group per bn_stats)
                for t in range(G):
                    _bn_stats_noopt(nc, st_sb[:, j0 + t, :], z16[:, t, :])

                # transpose each [128i, 128z] -> [128z, 128i] (f16, one bank)
                ztb = zpsum.tile([128, G, 128], F16, tag="ztb")
                for t in range(G):
                    nc.tensor.transpose(ztb[:, t, :], z16[:, t, :], ident16)
                zt_sb = ztpool.tile([128, G, 128], F16, tag="zt")
                nc.any.tensor_copy(zt_sb, ztb)

                # P_raw[i, h] per j, packed 32 j per psum bank
                if jg % 4 == 0:
                    pbank = ppsum.tile([128, JB, H], F32, tag="pbank")
                for t in range(G):
                    jj = (jg % 4) * G + t
                    nc.tensor.matmul(pbank[:, jj, :], zt_sb[:, t, :], wz_sb,
                                     start=True, stop=True)
                if jg % 4 == 3:
                    b = jg // 4
                    nc.any.tensor_copy(
                        P16[:, JB * b:JB * (b + 1), :], pbank)

            # stats finalize per round + corrections per bank
            for r in range(NR):
                jr = slice(RND * r, RND * (r + 1))
                st1 = st_sb[:, jr, 1:2]
                st2 = st_sb[:, jr, 2:3]
                st4 = st_sb[:, jr, 4:5]
                st5 = st_sb[:, jr, 5:6]
                mu2r = mu2[:, jr].rearrange("p (n o) -> p n o", o=1)
                rsr = rs[:, jr].rearrange("p (n o) -> p n o", o=1)
                mursr = murs[:, jr].rearrange("p (n o) -> p n o", o=1)
                nc.vector.tensor_tensor(out=mu2r, in0=st1, in1=st4, op=AL.add)
                dl = stmp.tile([128, RND, 1], F32, tag="dl")
                nc.vector.tensor_tensor(out=dl, in0=st1, in1=st4, op=AL.subtract)
                dh = stmp.tile([128, RND, 1], F32, tag="dh")
                nc.vector.tensor_scalar_mul(dh, dl, 0.5)
                q4 = stmp.tile([128, RND, 1], F32, tag="q4")
                nc.vector.tensor_tensor(out=q4, in0=dh, in1=dh, op=AL.mult)
                ve = stmp.tile([128, RND, 1], F32, tag="ve")
                nc.vector.tensor_tensor(out=ve, in0=st2, in1=st5, op=AL.add)
                veps = stmp.tile([128, RND, 1], F32, tag="veps")
                nc.vector.scalar_tensor_tensor(
                    out=veps, in0=ve, scalar=1.0 / DZ, in1=q4,
                    op0=AL.mult, op1=AL.add)
                sq = stmp.tile([128, RND, 1], F32, tag="sq")
                nc.scalar.activation(sq, veps, AF.Sqrt, bias=eps_sb)
                nc.vector.reciprocal(rsr, sq)
                nc.vector.tensor_tensor(out=mursr, in0=mu2r, in1=rsr, op=AL.mult)

            for b in range(NB):
                jb = slice(JB * b, JB * (b + 1))
                rs_rep = _bcast(rs[:, jb], [list(rs.ap[0]), [1, JB], [0, H]])
                murs_rep = _bcast(murs[:, jb], [list(murs.ap[0]), [1, JB], [0, H]])
                c1_rep = _bcast(c1h_sb, [list(c1h_sb.ap[0]), [0, JB], [1, H]])
                t1 = stmp.tile([128, JB, H], F32, tag="t1")
                nc.vector.tensor_tensor(out=t1, in0=P16[:, jb, :], in1=rs_rep,
                                        op=AL.mult)
                t2 = stmp.tile([128, JB, H], F32, tag="t2")
                nc.vector.tensor_tensor(out=t2, in0=murs_rep, in1=c1_rep,
                                        op=AL.mult)
                nc.vector.tensor_tensor(out=P16[:, jb, :], in0=t1, in1=t2,
                                        op=AL.subtract)

        # ---------------- Phase C: attention ----------------
        with (
            tc.tile_pool(name="scps", bufs=2, space="PSUM") as scps,
            tc.tile_pool(name="atps", bufs=2, space="PSUM") as atps,
            tc.tile_pool(name="ops", bufs=1, space="PSUM") as ops,
            tc.tile_pool(name="attn", bufs=2) as attnp,
        ):
            ob = ops.tile([128, 2, 8, HD], F32)
            for h in range(H):
                m, p0 = h // 2, 64 * (h % 2)
                scp = scps.tile([128, 2, 512], F32, tag="scp")
                for n in range(2):
                    nc.tensor.matmul(scp[:, n, :],
                                     qT_sb[p0:p0 + 64, m, :],
                                     kT_sb[p0:p0 + 64, m, 512 * n:512 * (n + 1)],
                                     start=True, stop=True)
                sc_sb = attnp.tile([128, S], F32, tag="sc")
                nc.vector.tensor_tensor(
                    out=sc_sb, in0=scp.rearrange("p a b -> p (a b)"),
                    in1=P16[:, :, h], op=AL.add)
                attn16 = attnp.tile([128, S], F16, tag="at")
                nc.scalar.activation(attn16, sc_sb, AF.Exp)
                nc.vector.tensor_reduce(
                    out=sums[:, h:h + 1], in_=attn16, axis=mybir.AxisListType.X,
                    op=AL.add)
                atb = atps.tile([128, G, 128], F16, tag="atb")
                for t in range(G):
                    nc.tensor.transpose(atb[:, t, :],
                                        attn16[:, 128 * t:128 * (t + 1)],
                                        ident16)
                attnT = attnp.tile([128, G, 128], F16, tag="atT")
                nc.any.tensor_copy(attnT, atb)
                for t in range(G):
                    nc.tensor.matmul(ob[:, h // 8, h % 8, :], attnT[:, t, :],
                                     v_sb[:, t, HD * h:HD * (h + 1)],
                                     start=(t == 0), stop=(t == G - 1))
                if h % 8 == 7:
                    hb = h // 8
                    nc.vector.reciprocal(inv[:, 8 * hb:8 * (hb + 1)],
                                         sums[:, 8 * hb:8 * (hb + 1)])
                    for hh in range(8 * hb, 8 * (hb + 1)):
                        nc.vector.scalar_tensor_tensor(
                            out=og16[:, HD * hh:HD * (hh + 1)],
                            in0=ob[:, hb, hh % 8, :],
                            scalar=inv[:, hh:hh + 1],
                            in1=g16[:, HD * hh:HD * (hh + 1)],
                            op0=AL.mult, op1=AL.mult)

        # ---------------- Phase D: output projection ----------------
        with (
            tc.tile_pool(name="wopool", bufs=1) as wopool,
            tc.tile_pool(name="dpsum", bufs=2, space="PSUM") as dpsum,
        ):
            wo_sb = wopool.tile([128, KT, DS], F16)
            nc.sync.dma_start(
                out=wo_sb, in_=woT16.rearrange("(m p) n -> p m n", p=128))
            ogb = dpsum.tile([128, G, 128], F16, tag="ogb")
            for t in range(G):
                nc.tensor.transpose(ogb[:, t, :],
                                    og16[:, 128 * t:128 * (t + 1)], ident16)
            nc.any.tensor_copy(ogT_sb.rearrange("p k n -> p (k n)"),
                               ogb.rearrange("p k n -> p (k n)"))
            for n in range(2):
                op_ = dpsum.tile([128, 512], F32, tag="op")
                for k in range(KT):
                    nc.tensor.matmul(op_, ogT_sb[:, k, :],
                                     wo_sb[:, k, 512 * n:512 * (n + 1)],
                                     start=(k == 0), stop=(k == KT - 1))
                nc.any.tensor_copy(out_sb[:, 512 * n:512 * (n + 1)], op_)
            nc.sync.dma_start(out=out_sh, in_=out_sb)


def prep_inputs(s, z, wq, bq, wk, wv, wg, z_norm_w, z_norm_b, wz, wo):
    """Host-side prep: shard + transpose/cast weights. Returns in_maps."""
    s2 = np.asarray(s)[0]                     # [S, DS]
    sT = np.ascontiguousarray(s2.T).astype(np.float16)
    wqT = np.ascontiguousarray((np.asarray(wq) / 8.0).T).astype(np.float16)
    wkT = np.ascontiguousarray(np.asarray(wk).T).astype(np.float16)
    wvT = np.ascontiguousarray(np.asarray(wv).T).astype(np.float16)
    wgT = np.ascontiguousarray(np.asarray(wg).T).astype(np.float16)
    woT = np.ascontiguousarray(np.asarray(wo).T).astype(np.float16)
    wz_f = (np.asarray(z_norm_w)[:, None] * np.asarray(wz).T)  # [DZ, H]
    wz16 = wz_f.astype(np.float16)
    # correction constant: c1_h = sum_z wz16[z, h]; folded x0.5 for mu2=2mu.
    # Use the f16-quantized wz so the correction matches the device P matmul.
    c1h = (0.5 * wz16.astype(np.float32).sum(axis=0))[None, :].astype(np.float32)
    bq8 = (np.asarray(bq) / 8.0).astype(np.float32)[:, None]
    z0 = np.asarray(z)[0]                     # [S, S, DZ]

    in_maps = []
    for c in range(NCORES):
        i0 = SI * c
        in_maps.append({
            "z_sh": np.ascontiguousarray(z0[i0:i0 + SI]).astype(np.float32),
            "sT16": sT,
            "sTi16": np.ascontiguousarray(sT[:, i0:i0 + SI]),
            "wqT16": wqT, "wkT16": wkT, "wvT16": wvT, "wgT16": wgT,
            "woT16": woT, "wz16": wz16, "c1h": c1h, "bq8": bq8,
        })
    return in_maps


_NC_CACHE = None


def _get_nc():
    global _NC_CACHE
    if _NC_CACHE is None:
        _NC_CACHE = build_nc()
    return _NC_CACHE


def kernel(**inputs):
    from concourse.bass_utils import run_bass_kernel_spmd
    nc = _get_nc()
    in_maps = prep_inputs(**inputs)
    res = run_bass_kernel_spmd(nc, in_maps, core_ids=list(range(NCORES)))
    out = np.empty((1, S, DS), dtype=np.float32)
    for c in range(NCORES):
        out[0, SI * c:SI * (c + 1), :] = res.results[c]["out_sh"]
    return out
